# revision 82
# baseline (speedup 1.0000x reference)
import sys

sys.path.insert(0, "/opt/trn_rl_repo")
import numpy as np
import ml_dtypes

from concourse import bacc, tile, mybir
from concourse.bass_utils import run_bass_kernel_spmd

BF16 = ml_dtypes.bfloat16
N_CORES = 8
N, K, C_IN, H, W = 512, 4, 3, 21, 21
HID, C2, ACT_DIM = 64, 16, 5
OBS_R = (H // 2, W // 2)
PIX = H * W          # 441
PIXP = 448           # padded pixel count (multiple of 8)
NR = PIXP // 8       # 56 rounds of 8 pixels
A_PC = N // N_CORES  # 64 agents per core
I_PC = A_PC * K      # 256 images per core

# packed bf16 weight blobs: A = w1s | w2b (needed at round 0), B = wc
WB_W1 = 0
WB_W2 = WB_W1 + 64
WB_COLS = WB_W2 + 32
WC_COLS = NR * ACT_DIM
# packed f32 blob columns: b1 | b2 | ey(5 rows)
WF_B1 = 0
WF_B2 = 1
WF_EY = 2
WF_COLS = WF_EY + ACT_DIM

_CACHE = {}
LAST_RESULT = None

import os as _os

SPL2 = int(_os.environ.get("K_SPL2", "318"))       # relu2 ACT els (of 512)


def _ensure_ntff_hook():
    """This image's antenv lacks axon_hooks; inject a shim so
    run_bass_kernel_spmd's trace path works (exec_time_ns)."""
    import types

    try:
        import antenv

        if hasattr(antenv, "axon_hooks"):
            return
        from trn_agent_boot.trn_boot import _ntff_profile_via_ctypes

        mod = types.ModuleType("antenv.axon_hooks")
        _h = [_ntff_profile_via_ctypes("/opt/axon/libaxon_pjrt.so")]
        mod.set_axon_ntff_profile_hook = lambda h: _h.__setitem__(0, h)
        mod.get_axon_ntff_profile_hook = lambda: _h[0]
        sys.modules["antenv.axon_hooks"] = mod
        antenv.axon_hooks = mod
    except Exception:
        pass


def _build_nc():
    f32 = mybir.dt.float32
    bf16 = mybir.dt.bfloat16
    RELU = mybir.ActivationFunctionType.Relu
    ADD = mybir.AluOpType.add
    MAX = mybir.AluOpType.max

    nc = bacc.Bacc("TRN2", target_bir_lowering=False, debug=False, num_devices=N_CORES)
    # xim: conv weights (WB_COLS) then NR rounds of 512, flat per partition
    xim = nc.declare_dram_parameter(
        "xim", [128, WB_COLS + NR * 2 * I_PC], bf16, isOutput=False
    )
    wcl = nc.declare_dram_parameter("wcl", [128, WC_COLS], bf16, isOutput=False)
    wfl = nc.declare_dram_parameter("wfl", [128, WF_COLS], f32, isOutput=False)
    mT4 = nc.declare_dram_parameter("mT4", [64, 4 * 128], bf16, isOutput=False)
    # output: this core's partial of Q in [5, 4, 128] layout (a, c, p)
    out = nc.declare_dram_parameter("out", [ACT_DIM, 4 * 128], f32, isOutput=True)

    with tile.TileContext(nc) as tc:
        with (
            tc.tile_pool(name="w", bufs=1) as wp,
            tc.tile_pool(name="x", bufs=3) as xp,
            tc.tile_pool(name="r", bufs=7) as rp,
            tc.tile_pool(name="pp", bufs=3) as pp,
            tc.tile_pool(name="sm", bufs=1) as sm,
            tc.tile_pool(name="psab", bufs=3, space="PSUM") as psab,
            tc.tile_pool(name="ps2", bufs=1, space="PSUM") as ps2,
            tc.tile_pool(name="psp", bufs=1, space="PSUM") as psp,
        ):
            # ---- DMA triggers cost ~600ns each, serialized per ring: put
            # xt0 first on sync, small weights next, and alternate the xim
            # group stream between the sync and gpsimd rings
            GROUPS = []
            g0 = 0
            for gl in (4, 8, 10, 10, 12, 12):
                GROUPS.append((g0, min(gl, NR - g0)))
                g0 += gl
            xt_tiles = []
            xt_of = {}
            for gi, (g0, gl) in enumerate(GROUPS):
                base = WB_COLS if gi == 0 else 0
                tg = "xt0" if gi == 0 else f"xt{gl}"
                xt = xp.tile([128, base + gl * 2 * I_PC], bf16, tag=tg)
                xt_tiles.append((xt, g0, gl))
                for R in range(g0, g0 + gl):
                    xt_of[R] = (xt, base + (R - g0) * 2 * I_PC)

            # preload the Relu ACT spline table while DMAs stream
            dumt = sm.tile([1, 2], f32, tag="dum")
            nc.gpsimd.memset(dumt[:], 0.0)
            nc.scalar.activation(dumt[:, 1:2], dumt[:, 0:1], RELU, bias=0.0)

            wf = wp.tile([128, WF_COLS], f32)
            nc.gpsimd.dma_start(wf[:], wfl[:])
            mt = wp.tile([64, 4 * 128], bf16)
            nc.gpsimd.dma_start(mt[:], mT4[:])
            wb = None
            wct = None
            for gi, (xt, g0, gl) in enumerate(xt_tiles):
                c0 = WB_COLS + g0 * 2 * I_PC
                c1 = WB_COLS + (g0 + gl) * 2 * I_PC
                if gi == 0:
                    # first group's DMA also carries the conv weights
                    wb = xt
                    nc.sync.dma_start(xt[:], xim[:, 0:c1])
                else:
                    nc.sync.dma_start(xt[:], xim[:, c0:c1])
                if gi == 1:
                    wct = wp.tile([128, WC_COLS], bf16)
                    nc.sync.dma_start(wct[:], wcl[:])

            w1t = wb[:, WB_W1 : WB_W1 + 64]
            w2t = wb[:, WB_W2 : WB_W2 + 32]
            b1t = wf[:, WF_B1 : WF_B1 + 1]
            b2t = wf[:, WF_B2 : WF_B2 + 1]
            eyt = wf[0:ACT_DIM, WF_EY : WF_EY + ACT_DIM]

            # projection accumulator, lives for the whole image loop
            qacc = psp.tile([ACT_DIM, I_PC], f32, tag="qacc")

            D = 2  # conv2 software-pipeline delay (rounds)
            prev = {}
            pend = None
            psC = None
            for R in range(NR + D):
                if R < NR:
                    xt, cb = xt_of[R]
                    # conv1: 4 row-strip matmuls into one 2-bank psum tile
                    pab = psab.tile([128, 2, 2 * I_PC], f32, tag="c1")
                    for s in range(4):
                        e = s % 2
                        nc.tensor.matmul(
                            pab[64 * e : 64 * (e + 1), s // 2, :],
                            w1t[32 * s : 32 * s + 27, :],
                            xt[32 * s : 32 * s + 27, cb : cb + 2 * I_PC],
                            start=True,
                            stop=True,
                            tile_position=(32 * s, 64 * e),
                        )
                    # relu1 + bias: one fat drain per round, alternating engine
                    rt = rp.tile([128, 2, 2 * I_PC], bf16, tag="r")
                    if R % 2 == 0:
                        nc.scalar.activation(rt[:], pab[:], RELU, bias=b1t[:, 0:1])
                    else:
                        nc.vector.tensor_scalar(
                            rt[:], pab[:], b1t[:, 0:1], 0.0, ADD, MAX
                        )
                    prev[R] = rt
                # conv2 for the pair {R-3, R-2}, issued back-to-back so the
                # next conv1 rounds' LDWs hide under conv1 (not conv2) MMs
                if R % 2 == 1 and R - 3 >= 0:
                    psC = ps2.tile([128, 2, I_PC], f32, tag="c2")
                    for Rp in (R - 3, R - 2):
                        if Rp >= NR:
                            continue
                        rtp = prev.pop(Rp)
                        for j in range(4):
                            nc.tensor.matmul(
                                psC[32 * j : 32 * (j + 1), Rp % 2, :],
                                w2t[:, :],
                                rtp[:, j % 2, (j // 2) * I_PC : (j // 2 + 1) * I_PC],
                                start=True,
                                stop=True,
                                tile_position=(0, 32 * j),
                            )
                    # relu2 + bias: whole-pair on one engine, alternating, to
                    # amortize the per-instruction constants
                    pt = pp.tile([128, 2, I_PC], bf16, tag="pt")
                    ptv = pt[:].rearrange("p b i -> p (b i)")
                    pcv = psC[:].rearrange("p b i -> p (b i)")
                    if ((R - 3) // 2) % 3 != 2:
                        nc.scalar.activation(ptv[:], pcv[:], RELU, bias=b2t[:, 0:1])
                    else:
                        nc.vector.tensor_scalar(
                            ptv[:], pcv[:], b2t[:, 0:1], 0.0, ADD, MAX
                        )
                    if pend is not None:
                        Pp, ptp = pend
                        for t in range(2):
                            Rq = 2 * Pp + t
                            nc.tensor.matmul(
                                qacc[:],
                                wct[:, Rq * ACT_DIM : (Rq + 1) * ACT_DIM],
                                ptp[:, t, :],
                                start=(Rq == 0),
                                stop=False,
                            )
                    pend = ((R - 3) // 2, pt)
            # flush the last pending projection pair
            Pp, ptp = pend
            for t in range(2):
                Rq = 2 * Pp + t
                nc.tensor.matmul(
                    qacc[:],
                    wct[:, Rq * ACT_DIM : (Rq + 1) * ACT_DIM],
                    ptp[:, t, :],
                    start=False,
                    stop=(Rq == NR - 1),
                )

            # ---- tail (fully local, no collectives):
            # fold K (mean folded into wc), transpose, mask partials, out DMA
            q5 = sm.tile([ACT_DIM, A_PC], f32, tag="q5")
            nc.vector.tensor_reduce(
                q5[:],
                qacc[:].rearrange("p (a k) -> p a k", k=K),
                mybir.AxisListType.X,
                ADD,
            )
            psT = ps2.tile([A_PC, ACT_DIM], f32, tag="c2")
            nc.tensor.matmul(psT[:], q5[:], eyt[:], start=True, stop=True)
            qT = sm.tile([A_PC, ACT_DIM], bf16, tag="qT")
            nc.vector.tensor_copy(qT[:], psT[:])

            # partial mask aggregation in ONE matmul (mask chunks contiguous),
            # transposed so the output DMA is 5 fat descriptors:
            # psM2[a, c*128+p] = sum_{j local} mask[128c+p, j] q[j, a]
            psM2 = ps2.tile([ACT_DIM, 4, 128], f32, tag="c2")
            nc.tensor.matmul(
                psM2[:].rearrange("a c p -> a (c p)"),
                qT[:],
                mt[:, 0 : 4 * 128],
                start=True,
                stop=True,
            )
            agi = sm.tile([ACT_DIM, 4 * 128], f32, tag="agi")
            pmv = psM2[:].rearrange("a c p -> a (c p)")
            nc.scalar.copy(agi[:, 0:256], pmv[:, 0:256])
            nc.vector.tensor_copy(agi[:, 256:512], pmv[:, 256:512])
            # two parallel half-DMAs on separate rings: the end-of-model
            # barrier waits on the last completion
            nc.gpsimd.dma_start(out[:, 0:256], agi[:, 0:256])
            nc.sync.dma_start(out[:, 256:512], agi[:, 256:512])

    nc.compile()
    return nc


def _host_prep(obs, action, state, conv1_w, conv1_b, conv2_w, conv2_b,
               obs_w, obs_b, act_w, act_b, val_w, val_b, adv_w, adv_b):
    f = np.float32
    obs = np.asarray(obs, f)
    action = np.asarray(action).astype(np.int64)
    state = np.asarray(state).astype(np.int64)
    conv1_w = np.asarray(conv1_w, f)
    conv1_b = np.asarray(conv1_b, f)
    conv2_w = np.asarray(conv2_w, f)
    conv2_b = np.asarray(conv2_b, f)
    obs_w = np.asarray(obs_w, f)
    obs_b = np.asarray(obs_b, f)
    act_w = np.asarray(act_w, f)
    act_b = np.asarray(act_b, f)
    val_w = np.asarray(val_w, f)
    val_b = np.asarray(val_b, f)
    adv_w = np.asarray(adv_w, f)
    adv_b = np.asarray(adv_b, f)

    # dueling head folded into a single linear: Q = latent @ Wq.T + bq
    Wq = val_w[0][None, :] + adv_w - adv_w.mean(axis=0)[None, :]  # [5, 32]
    bq = val_b[0] + adv_b - adv_b.mean()                          # [5]
    Wqo, Wqa = Wq[:, :16], Wq[:, 16:]
    W_combo = (Wqo @ obs_w) / K                                   # [5, 7056]

    aoh = np.zeros((N, ACT_DIM), f)
    aoh[np.arange(N), action] = 1.0
    a_enc = aoh @ act_w.T + act_b                                 # [512, 16]
    h = obs_b @ Wqo.T + a_enc @ Wqa.T                             # [512, 5]

    d = np.abs(state[:, None, :] - state[None, :, :])
    within = (d[..., 0] <= OBS_R[0]) & (d[..., 1] <= OBS_R[1])
    upper = np.triu(np.ones((N, N), bool), 1)
    mask = (np.eye(N, dtype=bool) | (within & upper)).astype(f)   # [512, 512]
    hbias = mask @ h + bq[None, :]                                # [512, 5]

    # device weight layouts
    w1 = conv1_w.reshape(HID, C_IN * 9)                           # [64, 27]
    w1s = np.zeros((128, 64), f)
    for s in range(4):
        w1s[32 * s : 32 * s + 27] = w1.T
    w2 = conv2_w.reshape(C2, HID)                                 # [16, 64]
    w2b = np.zeros((128, 32), f)
    w2b[0:64, 0:16] = w2.T
    w2b[64:128, 16:32] = w2.T
    Wc3 = W_combo.reshape(ACT_DIM, C2, PIX)                       # [5, 16, 441]
    wcf = np.zeros((128, NR, ACT_DIM), f)
    for G in range(NR):
        for q in range(8):
            p = 8 * G + q
            if p < PIX:
                wcf[16 * q : 16 * (q + 1), G, :] = Wc3[:, :, p].T
    wc = wcf.reshape(128, NR * ACT_DIM)

    # im2col: K27[(c,dh,dw), pix, img]
    from numpy.lib.stride_tricks import sliding_window_view

    obs_im = obs.reshape(N * K, C_IN, H, W)
    obs_p = np.pad(obs_im, ((0, 0), (0, 0), (1, 1), (1, 1)))
    win = sliding_window_view(obs_p, (3, 3), axis=(2, 3))         # [NK, 3, 21, 21, 3, 3]
    K27 = win.transpose(1, 4, 5, 2, 3, 0).reshape(27, PIX, N * K)
    K27p = np.zeros((27, PIXP, N * K), f)
    K27p[:, :PIX] = K27
    Kv = K27p.reshape(27, NR, 2, 4, N * K)                        # (k, R, h, s, img)

    wfl = np.zeros((128, WF_COLS), f)
    wfl[:, WF_B1] = np.tile(conv1_b, 2)
    wfl[:, WF_B2] = np.tile(conv2_b, 8)
    wfl[0:ACT_DIM, WF_EY : WF_EY + ACT_DIM] = np.eye(ACT_DIM, dtype=f)

    in_maps = []
    for r in range(N_CORES):
        i0, i1 = r * I_PC, (r + 1) * I_PC
        ximr = np.zeros((128, NR, 2 * I_PC), BF16)
        for s in range(4):
            blk = Kv[:, :, :, s, i0:i1].reshape(27, NR, 2 * I_PC)
            ximr[32 * s : 32 * s + 27, :, :] = blk.astype(BF16)
        a0 = r * A_PC
        # mask chunks as lhsT [64 local j, 4 blocks c, 128 global i]
        mcols = mask[:, a0 : a0 + A_PC]                            # [512 i, 64 j]
        mT4 = np.ascontiguousarray(
            mcols.T.reshape(A_PC, 4 * 128)
        )
        wbl = np.zeros((128, WB_COLS), f)
        wbl[:, WB_W1 : WB_W1 + 64] = w1s
        wbl[:, WB_W2 : WB_W2 + 32] = w2b
        ximf = np.concatenate(
            [wbl.astype(BF16), ximr.reshape(128, NR * 2 * I_PC)], axis=1
        )
        in_maps.append(
            {
                "xim": np.ascontiguousarray(ximf),
                "wcl": wc.astype(BF16),
                "wfl": wfl,
                "mT4": mT4.astype(BF16),
            }
        )
    return in_maps, hbias


def kernel(**inputs):
    global LAST_RESULT
    _ensure_ntff_hook()
    in_maps, hbias = _host_prep(**inputs)
    if "nc" not in _CACHE:
        _CACHE["nc"] = _build_nc()
    nc = _CACHE["nc"]
    res = run_bass_kernel_spmd(nc, in_maps, core_ids=list(range(N_CORES)))
    LAST_RESULT = res
    # each core holds its partial of Q in [5, 4, 128] (a, c, p) layout;
    # unshard = sum partials over cores (contraction-dim sharding) + host bias
    acc = np.zeros((ACT_DIM, 4 * 128), np.float32)
    for r in range(N_CORES):
        acc += res.results[r]["out"].astype(np.float32)
    outp = acc.reshape(ACT_DIM, N).T + hbias
    return outp.astype(np.float32)


# revision 83
# speedup vs baseline: 1.0520x; 1.0520x over previous
import sys

sys.path.insert(0, "/opt/trn_rl_repo")
import numpy as np
import ml_dtypes

from concourse import bacc, tile, mybir
from concourse.bass_utils import run_bass_kernel_spmd

BF16 = ml_dtypes.bfloat16
N_CORES = 8
N, K, C_IN, H, W = 512, 4, 3, 21, 21
HID, C2, ACT_DIM = 64, 16, 5
OBS_R = (H // 2, W // 2)
PIX = H * W          # 441
PIXP = 448           # padded pixel count (multiple of 8)
NR = PIXP // 8       # 56 rounds of 8 pixels
A_PC = N // N_CORES  # 64 agents per core
I_PC = A_PC * K      # 256 images per core

# packed bf16 weight blobs: A = w1s | w2b (needed at round 0), B = wc
WB_W1 = 0
WB_W2 = WB_W1 + 64
WB_COLS = WB_W2 + 32
WC_COLS = NR * ACT_DIM
# packed f32 blob columns: b1 | b2 | ey(5 rows)
WF_B1 = 0
WF_B2 = 1
WF_EY = 2
WF_COLS = WF_EY + ACT_DIM

_CACHE = {}
LAST_RESULT = None

import os as _os

SPL2 = int(_os.environ.get("K_SPL2", "318"))       # relu2 ACT els (of 512)


def _ensure_ntff_hook():
    """This image's antenv lacks axon_hooks; inject a shim so
    run_bass_kernel_spmd's trace path works (exec_time_ns)."""
    import types

    try:
        import antenv

        if hasattr(antenv, "axon_hooks"):
            return
        from trn_agent_boot.trn_boot import _ntff_profile_via_ctypes

        mod = types.ModuleType("antenv.axon_hooks")
        _h = [_ntff_profile_via_ctypes("/opt/axon/libaxon_pjrt.so")]
        mod.set_axon_ntff_profile_hook = lambda h: _h.__setitem__(0, h)
        mod.get_axon_ntff_profile_hook = lambda: _h[0]
        sys.modules["antenv.axon_hooks"] = mod
        antenv.axon_hooks = mod
    except Exception:
        pass


def _build_nc():
    f32 = mybir.dt.float32
    bf16 = mybir.dt.bfloat16
    RELU = mybir.ActivationFunctionType.Relu
    ADD = mybir.AluOpType.add
    MAX = mybir.AluOpType.max

    nc = bacc.Bacc("TRN2", target_bir_lowering=False, debug=False, num_devices=N_CORES)
    # xim: conv weights (WB_COLS) then NR rounds of 512, flat per partition
    xim = nc.declare_dram_parameter(
        "xim", [128, WB_COLS + NR * 2 * I_PC], bf16, isOutput=False
    )
    wcl = nc.declare_dram_parameter("wcl", [128, WC_COLS], bf16, isOutput=False)
    wfl = nc.declare_dram_parameter("wfl", [128, WF_COLS], f32, isOutput=False)
    mT4 = nc.declare_dram_parameter("mT4", [64, 4 * 128], bf16, isOutput=False)
    # output: this core's partial of Q in [5, 4, 128] layout (a, c, p)
    out = nc.declare_dram_parameter("out", [ACT_DIM, 4 * 128], f32, isOutput=True)

    with tile.TileContext(nc) as tc:
        with (
            tc.tile_pool(name="w", bufs=1) as wp,
            tc.tile_pool(name="x", bufs=3) as xp,
            tc.tile_pool(name="r", bufs=7) as rp,
            tc.tile_pool(name="pp", bufs=3) as pp,
            tc.tile_pool(name="sm", bufs=1) as sm,
            tc.tile_pool(name="psab", bufs=3, space="PSUM") as psab,
            tc.tile_pool(name="ps2", bufs=1, space="PSUM") as ps2,
            tc.tile_pool(name="psp", bufs=1, space="PSUM") as psp,
        ):
            # ---- DMA triggers cost ~600ns each, serialized per ring: put
            # xt0 first on sync, small weights next, and alternate the xim
            # group stream between the sync and gpsimd rings
            GROUPS = []
            g0 = 0
            for gl in (2, 4, 8, 10, 10, 12, 10):
                GROUPS.append((g0, min(gl, NR - g0)))
                g0 += gl
            xt_tiles = []
            xt_of = {}
            for gi, (g0, gl) in enumerate(GROUPS):
                base = WB_COLS if gi == 0 else 0
                tg = "xt0" if gi == 0 else f"xt{gl}"
                xt = xp.tile([128, base + gl * 2 * I_PC], bf16, tag=tg)
                xt_tiles.append((xt, g0, gl))
                for R in range(g0, g0 + gl):
                    xt_of[R] = (xt, base + (R - g0) * 2 * I_PC)

            # preload the Relu ACT spline table while DMAs stream
            dumt = sm.tile([1, 2], f32, tag="dum")
            nc.gpsimd.memset(dumt[:], 0.0)
            nc.scalar.activation(dumt[:, 1:2], dumt[:, 0:1], RELU, bias=0.0)

            wf = wp.tile([128, WF_COLS], f32)
            nc.gpsimd.dma_start(wf[:], wfl[:])
            mt = wp.tile([64, 4 * 128], bf16)
            nc.gpsimd.dma_start(mt[:], mT4[:])
            wb = None
            wct = None
            for gi, (xt, g0, gl) in enumerate(xt_tiles):
                c0 = WB_COLS + g0 * 2 * I_PC
                c1 = WB_COLS + (g0 + gl) * 2 * I_PC
                if gi == 0:
                    # first group's DMA also carries the conv weights
                    wb = xt
                    nc.sync.dma_start(xt[:], xim[:, 0:c1])
                else:
                    nc.sync.dma_start(xt[:], xim[:, c0:c1])
                if gi == 1:
                    wct = wp.tile([128, WC_COLS], bf16)
                    nc.sync.dma_start(wct[:], wcl[:])

            w1t = wb[:, WB_W1 : WB_W1 + 64]
            w2t = wb[:, WB_W2 : WB_W2 + 32]
            b1t = wf[:, WF_B1 : WF_B1 + 1]
            b2t = wf[:, WF_B2 : WF_B2 + 1]
            eyt = wf[0:ACT_DIM, WF_EY : WF_EY + ACT_DIM]

            # projection accumulator, lives for the whole image loop
            qacc = psp.tile([ACT_DIM, I_PC], f32, tag="qacc")

            D = 2  # conv2 software-pipeline delay (rounds)
            prev = {}
            pend = None
            psC = None
            for R in range(NR + D):
                if R < NR:
                    xt, cb = xt_of[R]
                    # conv1: 4 row-strip matmuls into one 2-bank psum tile
                    pab = psab.tile([128, 2, 2 * I_PC], f32, tag="c1")
                    for s in range(4):
                        e = s % 2
                        nc.tensor.matmul(
                            pab[64 * e : 64 * (e + 1), s // 2, :],
                            w1t[32 * s : 32 * s + 27, :],
                            xt[32 * s : 32 * s + 27, cb : cb + 2 * I_PC],
                            start=True,
                            stop=True,
                            tile_position=(32 * s, 64 * e),
                        )
                    # relu1 + bias: one fat drain per round, alternating engine
                    rt = rp.tile([128, 2, 2 * I_PC], bf16, tag="r")
                    if R % 2 == 0:
                        nc.scalar.activation(rt[:], pab[:], RELU, bias=b1t[:, 0:1])
                    else:
                        nc.vector.tensor_scalar(
                            rt[:], pab[:], b1t[:, 0:1], 0.0, ADD, MAX
                        )
                    prev[R] = rt
                # conv2 for the pair {R-3, R-2}, issued back-to-back so the
                # next conv1 rounds' LDWs hide under conv1 (not conv2) MMs
                if R % 2 == 1 and R - 3 >= 0:
                    psC = ps2.tile([128, 2, I_PC], f32, tag="c2")
                    for Rp in (R - 3, R - 2):
                        if Rp >= NR:
                            continue
                        rtp = prev.pop(Rp)
                        for j in range(4):
                            nc.tensor.matmul(
                                psC[32 * j : 32 * (j + 1), Rp % 2, :],
                                w2t[:, :],
                                rtp[:, j % 2, (j // 2) * I_PC : (j // 2 + 1) * I_PC],
                                start=True,
                                stop=True,
                                tile_position=(0, 32 * j),
                            )
                    # relu2 + bias: whole-pair on one engine, alternating, to
                    # amortize the per-instruction constants
                    pt = pp.tile([128, 2, I_PC], bf16, tag="pt")
                    ptv = pt[:].rearrange("p b i -> p (b i)")
                    pcv = psC[:].rearrange("p b i -> p (b i)")
                    if ((R - 3) // 2) % 3 != 2:
                        nc.scalar.activation(ptv[:], pcv[:], RELU, bias=b2t[:, 0:1])
                    else:
                        nc.vector.tensor_scalar(
                            ptv[:], pcv[:], b2t[:, 0:1], 0.0, ADD, MAX
                        )
                    if pend is not None:
                        Pp, ptp = pend
                        for t in range(2):
                            Rq = 2 * Pp + t
                            nc.tensor.matmul(
                                qacc[:],
                                wct[:, Rq * ACT_DIM : (Rq + 1) * ACT_DIM],
                                ptp[:, t, :],
                                start=(Rq == 0),
                                stop=False,
                            )
                    pend = ((R - 3) // 2, pt)
            # flush the last pending projection pair
            Pp, ptp = pend
            for t in range(2):
                Rq = 2 * Pp + t
                nc.tensor.matmul(
                    qacc[:],
                    wct[:, Rq * ACT_DIM : (Rq + 1) * ACT_DIM],
                    ptp[:, t, :],
                    start=False,
                    stop=(Rq == NR - 1),
                )

            # ---- tail (fully local, no collectives):
            # fold K (mean folded into wc), transpose, mask partials, out DMA
            q5 = sm.tile([ACT_DIM, A_PC], f32, tag="q5")
            nc.vector.tensor_reduce(
                q5[:],
                qacc[:].rearrange("p (a k) -> p a k", k=K),
                mybir.AxisListType.X,
                ADD,
            )
            psT = ps2.tile([A_PC, ACT_DIM], f32, tag="c2")
            nc.tensor.matmul(psT[:], q5[:], eyt[:], start=True, stop=True)
            qT = sm.tile([A_PC, ACT_DIM], bf16, tag="qT")
            nc.vector.tensor_copy(qT[:], psT[:])

            # partial mask aggregation in ONE matmul (mask chunks contiguous),
            # transposed so the output DMA is 5 fat descriptors:
            # psM2[a, c*128+p] = sum_{j local} mask[128c+p, j] q[j, a]
            psM2 = ps2.tile([ACT_DIM, 4, 128], f32, tag="c2")
            nc.tensor.matmul(
                psM2[:].rearrange("a c p -> a (c p)"),
                qT[:],
                mt[:, 0 : 4 * 128],
                start=True,
                stop=True,
            )
            agi = sm.tile([ACT_DIM, 4 * 128], f32, tag="agi")
            pmv = psM2[:].rearrange("a c p -> a (c p)")
            nc.scalar.copy(agi[:, 0:256], pmv[:, 0:256])
            nc.vector.tensor_copy(agi[:, 256:512], pmv[:, 256:512])
            # two parallel half-DMAs on separate rings: the end-of-model
            # barrier waits on the last completion
            nc.gpsimd.dma_start(out[:, 0:256], agi[:, 0:256])
            nc.sync.dma_start(out[:, 256:512], agi[:, 256:512])

    nc.compile()
    return nc


def _host_prep(obs, action, state, conv1_w, conv1_b, conv2_w, conv2_b,
               obs_w, obs_b, act_w, act_b, val_w, val_b, adv_w, adv_b):
    f = np.float32
    obs = np.asarray(obs, f)
    action = np.asarray(action).astype(np.int64)
    state = np.asarray(state).astype(np.int64)
    conv1_w = np.asarray(conv1_w, f)
    conv1_b = np.asarray(conv1_b, f)
    conv2_w = np.asarray(conv2_w, f)
    conv2_b = np.asarray(conv2_b, f)
    obs_w = np.asarray(obs_w, f)
    obs_b = np.asarray(obs_b, f)
    act_w = np.asarray(act_w, f)
    act_b = np.asarray(act_b, f)
    val_w = np.asarray(val_w, f)
    val_b = np.asarray(val_b, f)
    adv_w = np.asarray(adv_w, f)
    adv_b = np.asarray(adv_b, f)

    # dueling head folded into a single linear: Q = latent @ Wq.T + bq
    Wq = val_w[0][None, :] + adv_w - adv_w.mean(axis=0)[None, :]  # [5, 32]
    bq = val_b[0] + adv_b - adv_b.mean()                          # [5]
    Wqo, Wqa = Wq[:, :16], Wq[:, 16:]
    W_combo = (Wqo @ obs_w) / K                                   # [5, 7056]

    aoh = np.zeros((N, ACT_DIM), f)
    aoh[np.arange(N), action] = 1.0
    a_enc = aoh @ act_w.T + act_b                                 # [512, 16]
    h = obs_b @ Wqo.T + a_enc @ Wqa.T                             # [512, 5]

    d = np.abs(state[:, None, :] - state[None, :, :])
    within = (d[..., 0] <= OBS_R[0]) & (d[..., 1] <= OBS_R[1])
    upper = np.triu(np.ones((N, N), bool), 1)
    mask = (np.eye(N, dtype=bool) | (within & upper)).astype(f)   # [512, 512]
    hbias = mask @ h + bq[None, :]                                # [512, 5]

    # device weight layouts
    w1 = conv1_w.reshape(HID, C_IN * 9)                           # [64, 27]
    w1s = np.zeros((128, 64), f)
    for s in range(4):
        w1s[32 * s : 32 * s + 27] = w1.T
    w2 = conv2_w.reshape(C2, HID)                                 # [16, 64]
    w2b = np.zeros((128, 32), f)
    w2b[0:64, 0:16] = w2.T
    w2b[64:128, 16:32] = w2.T
    Wc3 = W_combo.reshape(ACT_DIM, C2, PIX)                       # [5, 16, 441]
    wcf = np.zeros((128, NR, ACT_DIM), f)
    for G in range(NR):
        for q in range(8):
            p = 8 * G + q
            if p < PIX:
                wcf[16 * q : 16 * (q + 1), G, :] = Wc3[:, :, p].T
    wc = wcf.reshape(128, NR * ACT_DIM)

    # im2col: K27[(c,dh,dw), pix, img]
    from numpy.lib.stride_tricks import sliding_window_view

    obs_im = obs.reshape(N * K, C_IN, H, W)
    obs_p = np.pad(obs_im, ((0, 0), (0, 0), (1, 1), (1, 1)))
    win = sliding_window_view(obs_p, (3, 3), axis=(2, 3))         # [NK, 3, 21, 21, 3, 3]
    K27 = win.transpose(1, 4, 5, 2, 3, 0).reshape(27, PIX, N * K)
    K27p = np.zeros((27, PIXP, N * K), f)
    K27p[:, :PIX] = K27
    Kv = K27p.reshape(27, NR, 2, 4, N * K)                        # (k, R, h, s, img)

    wfl = np.zeros((128, WF_COLS), f)
    wfl[:, WF_B1] = np.tile(conv1_b, 2)
    wfl[:, WF_B2] = np.tile(conv2_b, 8)
    wfl[0:ACT_DIM, WF_EY : WF_EY + ACT_DIM] = np.eye(ACT_DIM, dtype=f)

    in_maps = []
    for r in range(N_CORES):
        i0, i1 = r * I_PC, (r + 1) * I_PC
        ximr = np.zeros((128, NR, 2 * I_PC), BF16)
        for s in range(4):
            blk = Kv[:, :, :, s, i0:i1].reshape(27, NR, 2 * I_PC)
            ximr[32 * s : 32 * s + 27, :, :] = blk.astype(BF16)
        a0 = r * A_PC
        # mask chunks as lhsT [64 local j, 4 blocks c, 128 global i]
        mcols = mask[:, a0 : a0 + A_PC]                            # [512 i, 64 j]
        mT4 = np.ascontiguousarray(
            mcols.T.reshape(A_PC, 4 * 128)
        )
        wbl = np.zeros((128, WB_COLS), f)
        wbl[:, WB_W1 : WB_W1 + 64] = w1s
        wbl[:, WB_W2 : WB_W2 + 32] = w2b
        ximf = np.concatenate(
            [wbl.astype(BF16), ximr.reshape(128, NR * 2 * I_PC)], axis=1
        )
        in_maps.append(
            {
                "xim": np.ascontiguousarray(ximf),
                "wcl": wc.astype(BF16),
                "wfl": wfl,
                "mT4": mT4.astype(BF16),
            }
        )
    return in_maps, hbias


def kernel(**inputs):
    global LAST_RESULT
    _ensure_ntff_hook()
    in_maps, hbias = _host_prep(**inputs)
    if "nc" not in _CACHE:
        _CACHE["nc"] = _build_nc()
    nc = _CACHE["nc"]
    res = run_bass_kernel_spmd(nc, in_maps, core_ids=list(range(N_CORES)))
    LAST_RESULT = res
    # each core holds its partial of Q in [5, 4, 128] (a, c, p) layout;
    # unshard = sum partials over cores (contraction-dim sharding) + host bias
    acc = np.zeros((ACT_DIM, 4 * 128), np.float32)
    for r in range(N_CORES):
        acc += res.results[r]["out"].astype(np.float32)
    outp = acc.reshape(ACT_DIM, N).T + hbias
    return outp.astype(np.float32)
